# revision 1
# baseline (speedup 1.0000x reference)
"""Trainium2 Bass kernel: GNN message passing (metasurface inverse design).

Distribution (8 NeuronCores):
  - Edges sharded by target-node range (12500 nodes per core).
  - Node state h replicated each layer via AllGather of the per-core shard.
  - Per-edge gather h[src] + per-node scatter-add use the SWDGE
    dma_gather / dma_scatter_add extended instructions (512B rows at
    near-line-rate).
  - Dense per-node work (matmuls, relu) computed on each core for its own
    node range only; readout MLP replicated after a tiny AllReduce.

Math restructuring (linearity of the edge matmul over the segment-sum):
    reference: agg = segsum_tgt(concat(h[src], h[tgt]) @ Wm[l] + bm[l])
    here:      G   = segsum_tgt(h[src])            (pure DMA gather+scatter)
               agg = G @ W1 + deg * (h @ W2) + deg x bm
  with Wm[l] = [W1; W2] and deg = in-degree.  This removes the per-edge
  (E x 2H x H) matmul entirely; all FLOPs become dense per-node matmuls.

Scatter-add duplicate-index safety: the q7 ucode assigns descriptor for
logical position p (within each 128-index chunk) to SDMA engine
  engine(p) = 2*(((p % 64) // 4) % 8) + p // 64.
Same-target edges are packed (host-side) into one engine class so their
HBM read-modify-writes serialize on a single engine's FIFO.
"""

import os

import numpy as np

import concourse.bacc as bacc
import concourse.bass as bass
import concourse.mybir as mybir
import concourse.tile as tile
from concourse.bass_utils import run_bass_kernel_spmd
from concourse.masks import make_identity

F32 = mybir.dt.float32
I16 = mybir.dt.int16
NCORES = 8
H = 128
P = 128
TWO_PI = 6.283185307179586

# logical position within a 128-index chunk -> SDMA engine (q7 swizzle tables)
_CLASS_SLOTS = [[] for _ in range(16)]
for _p in range(128):
    _CLASS_SLOTS[2 * (((_p % 64) // 4) % 8) + _p // 64].append(_p)
_CLASS_SLOTS = np.array(_CLASS_SLOTS, dtype=np.int64)  # [16, 8]


class Cfg:
    def __init__(self, N, E, B, M, L, chunk, blk=8192):
        assert N % NCORES == 0
        self.N, self.E, self.B, self.M, self.L = N, E, B, M, L
        self.chunk = chunk
        self.blk = blk
        self.ns = N // NCORES
        ns_pad = ((self.ns + 511) // 512) * 512
        if ns_pad == self.ns:
            ns_pad += 512  # need trash rows above ns for scatter padding
        self.ns_pad = ns_pad
        self.nchunk = -(-N // chunk)
        assert chunk < 32768 and self.ns_pad < 32768


def _pack_edges(cfg, src, tgt):
    """Round-based per-core int16 gather/scatter index layout.

    Each scatter-add call must contain at most ONE edge per target node:
    the SDMA engines pipeline the CCE read-modify-write, so duplicate
    targets within one call lose updates (verified on HW).  Edge j of a
    target goes to round j; calls are (round x src-chunk-group) slices and
    are serialized against each other by Tile's WAW tracking on agg.

    Returns gidx, sidx: [NCORES, 128, TOT] int16 and calls:
    list of (col0, nidx, parity, [(chunk, gcol0, gnidx), ...]).
    """
    ns, chunk, nck = cfg.ns, cfg.chunk, cfg.nchunk
    core = tgt // ns
    lt = (tgt - core * ns).astype(np.int64)
    k = src // chunk
    ls = (src - k * chunk).astype(np.int64)

    # round = rank of edge within its target
    o1 = np.lexsort((k, tgt))
    cnt_t = np.bincount(tgt, minlength=cfg.N)
    st_t = np.concatenate([[0], np.cumsum(cnt_t)[:-1]])
    r = np.empty(len(tgt), np.int64)
    r[o1] = np.arange(len(tgt)) - st_t[tgt[o1]]
    NR = int(r.max()) + 1

    # position within (core, round, chunk), ordered by lt
    key2 = (core * NR + r) * nck + k
    o2 = np.lexsort((lt, key2))
    ng2 = NCORES * NR * nck
    cnt2 = np.bincount(key2, minlength=ng2)
    st2 = np.concatenate([[0], np.cumsum(cnt2)[:-1]])
    pos = np.empty(len(tgt), np.int64)
    pos[o2] = np.arange(len(tgt)) - st2[key2[o2]]

    # segment sizes: max over cores, padded to 128 slots
    n_rk = cnt2.reshape(NCORES, NR, nck).max(axis=0)
    S_rk = ((n_rk + 127) // 128) * 128
    S_rk[(n_rk == 0)] = 0

    # segment slot offsets + call grouping (<= cap slots per scatter call)
    cap = cfg.blk
    seg_off = np.full((NR, nck), -1, np.int64)
    calls = []
    cur = 0
    pcount = 0
    for rr in range(NR):
        segs = [[kk, int(S_rk[rr, kk])] for kk in range(nck) if S_rk[rr, kk] > 0]
        i = 0
        while i < len(segs):
            c0 = cur
            gathers = []
            while i < len(segs):
                kk, sz = segs[i]
                room = cap - (cur - c0)
                if room <= 0:
                    break
                take = min(sz, room)
                if seg_off[rr, kk] < 0:
                    seg_off[rr, kk] = cur
                gathers.append((kk, cur // 16, take))
                cur += take
                if take < sz:
                    segs[i][1] = sz - take
                    break
                i += 1
            calls.append((c0 // 16, cur - c0, pcount % 2, gathers))
            pcount += 1
    TOT = cur // 16

    gpos = seg_off[r, k] + pos

    gidx = np.zeros((NCORES, 16, TOT), np.int16)
    trash = cfg.ns + (np.arange(16)[:, None] + 16 * np.arange(TOT)[None, :]) % (
        cfg.ns_pad - cfg.ns
    )
    sidx = np.broadcast_to(trash.astype(np.int16), (NCORES, 16, TOT)).copy()
    gidx[core, gpos % 16, gpos // 16] = ls.astype(np.int16)
    sidx[core, gpos % 16, gpos // 16] = lt.astype(np.int16)

    gidx = np.tile(gidx, (1, 8, 1))
    sidx = np.tile(sidx, (1, 8, 1))
    return gidx, sidx, calls


def _build(cfg, TOT, calls):
    """Emit the Bass/Tile program (identical for all cores)."""
    phase = os.environ.get("GNN_PHASE", "full")
    NS, NSP, B, M, L = cfg.ns, cfg.ns_pad, cfg.B, cfg.M, cfg.L
    NT = NSP // 512  # dense tiles (512 nodes each)
    NBLK = NSP // 128  # 128-node blocks
    MT = M // 128

    nc = bacc.Bacc(None, num_devices=NCORES)

    x_s = nc.dram_tensor("x_s", [NSP, H], F32, kind="ExternalInput")
    Wemb = nc.dram_tensor("Wemb", [H, H], F32, kind="ExternalInput")
    bemb = nc.dram_tensor("bemb", [H, 1], F32, kind="ExternalInput")
    Wm = nc.dram_tensor("Wm", [L, 2 * H, H], F32, kind="ExternalInput")
    bmv = nc.dram_tensor("bmv", [L, 1, H], F32, kind="ExternalInput")
    W1r = nc.dram_tensor("W1r", [H, 2 * H], F32, kind="ExternalInput")
    b1r = nc.dram_tensor("b1r", [H, 2], F32, kind="ExternalInput")
    W2r = nc.dram_tensor("W2r", [2 * H, M], F32, kind="ExternalInput")
    b2r = nc.dram_tensor("b2r", [H, MT], F32, kind="ExternalInput")
    gidx = nc.dram_tensor("gidx", [P, TOT], I16, kind="ExternalInput")
    sidx = nc.dram_tensor("sidx", [P, TOT], I16, kind="ExternalInput")
    deg_pb = nc.dram_tensor("deg_pb", [P, NBLK], F32, kind="ExternalInput")
    degT = nc.dram_tensor("degT", [1, NSP], F32, kind="ExternalInput")
    ind = nc.dram_tensor("ind", [P, NBLK * B], F32, kind="ExternalInput")

    outT = nc.dram_tensor("outT", [M, B], F32, kind="ExternalOutput")

    h_full = nc.dram_tensor("h_full", [cfg.N, H], F32, addr_space="Shared")
    h_sh = nc.dram_tensor("h_sh", [NSP, H], F32)
    aggA = nc.dram_tensor("aggA", [NSP, H], F32)
    aggB = nc.dram_tensor("aggB", [NSP, H], F32)
    gpart = nc.dram_tensor("gpart", [H, B], F32)
    gsum = nc.dram_tensor("gsum", [H, B], F32, addr_space="Shared")

    groups = [list(range(NCORES))]

    with tile.TileContext(nc) as tc:
        with (
            tc.tile_pool(name="const", bufs=1) as cp,
            tc.tile_pool(name="work", bufs=2) as wp,
            tc.tile_pool(name="edge", bufs=2) as ep,
        ):
            pp_cm = tc.tile_pool(name="psum", bufs=2, space="PSUM")
            pp = pp_cm.__enter__()
            # ---- persistent constants in SBUF ----
            ident = cp.tile([P, P], F32, tag="ident")
            make_identity(nc, ident[:])
            Wemb_sb = cp.tile([H, H], F32, tag="wemb")
            nc.sync.dma_start(Wemb_sb[:], Wemb[:])
            bemb_sb = cp.tile([H, 1], F32, tag="bemb")
            nc.sync.dma_start(bemb_sb[:], bemb[:])
            W1_sb, W2_sb, bm_sb = [], [], []
            for l in range(L):
                w1 = cp.tile([H, H], F32, tag=f"w1_{l}")
                nc.sync.dma_start(w1[:], Wm[l, :H, :])
                w2 = cp.tile([H, H], F32, tag=f"w2_{l}")
                nc.sync.dma_start(w2[:], Wm[l, H:, :])
                bm_t = cp.tile([1, H], F32, tag=f"bm_{l}")
                nc.sync.dma_start(bm_t[:], bmv[l, :, :])
                W1_sb.append(w1)
                W2_sb.append(w2)
                bm_sb.append(bm_t)
            gidx_sb = cp.tile([P, TOT], I16, tag="gidx")
            nc.sync.dma_start(gidx_sb[:], gidx[:])
            sidx_sb = cp.tile([P, TOT], I16, tag="sidx")
            nc.sync.dma_start(sidx_sb[:], sidx[:])
            deg_sb = cp.tile([P, NBLK], F32, tag="deg")
            nc.sync.dma_start(deg_sb[:], deg_pb[:])
            zero_sb = cp.tile([P, 1600], F32, tag="zero")
            nc.vector.memset(zero_sb[:], 0.0)

            def load_nm(pool, dram_rows, tag):
                """Load 512 node-major rows as [128, 4, 128] (block, feat)."""
                t = pool.tile([P, 4 * H], F32, tag=tag)
                nc.sync.dma_start(
                    t[:].rearrange("p (b f) -> p b f", f=H),
                    dram_rows.rearrange("(b p) f -> p b f", p=P),
                )
                return t

            def store_nm(t, dram_rows):
                nc.sync.dma_start(
                    dram_rows.rearrange("(b p) f -> p b f", p=P),
                    t[:].rearrange("p (b f) -> p b f", f=H),
                )

            def transpose4(src_sb, tag):
                """4x PE transpose of [128,(4,128)] blocks -> PSUM [128,512]."""
                ps = pp.tile([P, 512], F32, tag=tag)
                for b in range(4):
                    nc.tensor.transpose(
                        ps[:, b * H : (b + 1) * H],
                        src_sb[:, b * H : (b + 1) * H],
                        ident[:],
                    )
                return ps

            # ---- embedding: h0 = x @ Wemb + bemb (own shard) ----
            for t in range(NT):
                rows = slice(512 * t, 512 * (t + 1))
                xt = load_nm(wp, x_s[rows, :], "in_a")
                ps_x = transpose4(xt, "ps_a")
                xT = wp.tile([P, 512], F32, tag="t_a")
                nc.vector.tensor_copy(xT[:], ps_x[:])
                ps_h = pp.tile([P, 512], F32, tag="ps_mm")
                nc.tensor.matmul(ps_h[:], Wemb_sb[:], xT[:], start=True, stop=True)
                hT = wp.tile([P, 512], F32, tag="t_b")
                nc.vector.tensor_scalar_add(hT[:], ps_h[:], bemb_sb[:, 0:1])
                ps_nm = transpose4(hT, "ps_nm")
                hn = wp.tile([P, 4 * H], F32, tag="out_a")
                nc.vector.tensor_copy(hn[:], ps_nm[:])
                store_nm(hn, h_sh[rows, :])

            nc.gpsimd.collective_compute(
                "AllGather",
                mybir.AluOpType.bypass,
                replica_groups=groups,
                ins=[h_sh[0:NS, :]],
                outs=[h_full[:, :]],
            )

            # ---- message-passing layers ----
            CAPROWS = max(nidx for _, nidx, _, _ in calls) // 128
            for l in range(L):
                # zero both agg buffers
                for ag in (aggA, aggB):
                    ztot = NSP * H
                    zc = 1600 * P
                    off = 0
                    while off < ztot:
                        n = min(zc, ztot - off)
                        nc.gpsimd.dma_start(
                            ag[:, :]
                            .rearrange("n f -> (n f)")[off : off + n]
                            .rearrange("(p m) -> p m", p=P),
                            zero_sb[:, : n // P],
                        )
                        off += n

                # gather h[src] -> scatter-add into agg (own tgt range).
                # each call has unique targets; WAW on the agg buffer
                # serializes same-buffer calls (RMW hazard across calls).
                for c0, nidx, par, gathers in ([] if phase == "noedge" else calls):
                    et = ep.tile([P, CAPROWS * H], F32, tag="edge")
                    ev = et[:].rearrange("p (c e) -> p c e", e=H)
                    for kk, gc0, gn in gathers:
                        r0 = (gc0 * 16 - c0 * 16) // 128
                        nc.gpsimd.dma_gather(
                            ev[:, r0 : r0 + gn // 128, :],
                            h_full[kk * cfg.chunk : (kk + 1) * cfg.chunk, :],
                            gidx_sb[:, gc0 : gc0 + gn // 16],
                            gn,
                            gn,
                            H,
                            single_packet=False,
                        )
                    nc.gpsimd.dma_scatter_add(
                        (aggA if par == 0 else aggB)[:, :],
                        ev[:, : nidx // 128, :],
                        sidx_sb[:, c0 : c0 + nidx // 16],
                        nidx,
                        nidx,
                        H,
                        single_packet=False,
                    )

                # dense: h = relu(h + G @ W1 + deg*(h @ W2) + deg x bm)
                for t in range(NT if phase != "nodense" else 0):
                    rows = slice(512 * t, 512 * (t + 1))
                    ga = load_nm(wp, aggA[rows, :], "in_a")
                    gb = load_nm(wp, aggB[rows, :], "in_c")
                    gt = wp.tile([P, 4 * H], F32, tag="in_g")
                    nc.vector.tensor_add(gt[:], ga[:], gb[:])
                    ht = load_nm(wp, h_sh[rows, :], "in_b")
                    degT_t = wp.tile([1, 512], F32, tag="degt")
                    nc.sync.dma_start(degT_t[:], degT[:, rows])
                    dh = wp.tile([P, 4 * H], F32, tag="dh")
                    for b in range(4):
                        bs = slice(b * H, (b + 1) * H)
                        nc.vector.tensor_scalar_mul(
                            dh[:, bs], ht[:, bs], deg_sb[:, 4 * t + b : 4 * t + b + 1]
                        )
                    ps_g = transpose4(gt, "ps_a")
                    gT = wp.tile([P, 512], F32, tag="t_a")
                    nc.vector.tensor_copy(gT[:], ps_g[:])
                    ps_d = transpose4(dh, "ps_b")
                    dhT = wp.tile([P, 512], F32, tag="t_b")
                    nc.vector.tensor_copy(dhT[:], ps_d[:])
                    ps_mm = pp.tile([P, 512], F32, tag="ps_mm")
                    nc.tensor.matmul(ps_mm[:], W1_sb[l][:], gT[:], start=True, stop=False)
                    nc.tensor.matmul(ps_mm[:], W2_sb[l][:], dhT[:], start=False, stop=False)
                    nc.tensor.matmul(ps_mm[:], bm_sb[l][:], degT_t[:], start=False, stop=True)
                    aggT = wp.tile([P, 512], F32, tag="t_c")
                    nc.vector.tensor_copy(aggT[:], ps_mm[:])
                    ps_nm = transpose4(aggT, "ps_nm")
                    hn = wp.tile([P, 4 * H], F32, tag="out_a")
                    nc.vector.tensor_add(hn[:], ht[:], ps_nm[:])
                    nc.vector.tensor_scalar_max(hn[:], hn[:], 0.0)
                    store_nm(hn, h_sh[rows, :])

                if l < L - 1:
                    nc.gpsimd.collective_compute(
                        "AllGather",
                        mybir.AluOpType.bypass,
                        replica_groups=groups,
                        ins=[h_sh[0:NS, :]],
                        outs=[h_full[:, :]],
                    )

            # ---- readout: g = per-graph mean (ind already holds 1/count) ----
            pp_cm.__exit__(None, None, None)
            pp_cm = tc.tile_pool(name="psum_ro", bufs=1, space="PSUM")
            pp = pp_cm.__enter__()
            ind_sb = cp.tile([P, NBLK * B], F32, tag="ind")
            nc.sync.dma_start(ind_sb[:], ind[:])
            ps_gr = pp.tile([P, B], F32, tag="ps_gr")
            for t in range(NT):
                rows = slice(512 * t, 512 * (t + 1))
                hro = load_nm(wp, h_sh[rows, :], "in_a")
                for b in range(4):
                    blk = 4 * t + b
                    nc.tensor.matmul(
                        ps_gr[:],
                        hro[:, b * H : (b + 1) * H],
                        ind_sb[:, blk * B : (blk + 1) * B],
                        start=(blk == 0),
                        stop=(blk == NBLK - 1),
                    )
            gp_sb = wp.tile([P, B], F32, tag="gp")
            nc.vector.tensor_copy(gp_sb[:], ps_gr[:])
            nc.sync.dma_start(gpart[:, :], gp_sb[:])
            nc.gpsimd.collective_compute(
                "AllReduce",
                mybir.AluOpType.add,
                replica_groups=groups,
                ins=[gpart[:, :]],
                outs=[gsum[:, :]],
            )
            gs_sb = wp.tile([P, B], F32, tag="gs")
            nc.sync.dma_start(gs_sb[:], gsum[:, :])

            W1r_sb = cp.tile([H, 2 * H], F32, tag="w1r")
            nc.sync.dma_start(W1r_sb[:], W1r[:])
            b1_sb = cp.tile([H, 2], F32, tag="b1r")
            nc.sync.dma_start(b1_sb[:], b1r[:])
            W2ra_sb = cp.tile([H, M], F32, tag="w2ra")
            nc.sync.dma_start(W2ra_sb[:], W2r[0:H, :])
            W2rb_sb = cp.tile([H, M], F32, tag="w2rb")
            nc.sync.dma_start(W2rb_sb[:], W2r[H:, :])
            b2_sb = cp.tile([H, MT], F32, tag="b2r")
            nc.sync.dma_start(b2_sb[:], b2r[:])

            z1 = []
            for i in range(2):
                ps_z = pp.tile([P, B], F32, tag=f"ps_z{i}")
                nc.tensor.matmul(
                    ps_z[:], W1r_sb[:, i * H : (i + 1) * H], gs_sb[:], start=True, stop=True
                )
                zt = wp.tile([P, B], F32, tag=f"z1_{i}")
                nc.vector.tensor_scalar(
                    zt[:], ps_z[:], b1_sb[:, i : i + 1], 0.0,
                    mybir.AluOpType.add, mybir.AluOpType.max,
                )
                z1.append(zt)

            o_sb = wp.tile([P, MT * B], F32, tag="o")
            for m in range(MT):
                ps_o = pp.tile([P, B], F32, tag="ps_o")
                ms = slice(m * H, (m + 1) * H)
                nc.tensor.matmul(ps_o[:], W2ra_sb[:, ms], z1[0][:], start=True, stop=False)
                nc.tensor.matmul(ps_o[:], W2rb_sb[:, ms], z1[1][:], start=False, stop=True)
                nc.scalar.activation(
                    o_sb[:, m * B : (m + 1) * B],
                    ps_o[:],
                    mybir.ActivationFunctionType.Sigmoid,
                    bias=b2_sb[:, m : m + 1],
                    scale=1.0,
                )
            nc.vector.tensor_scalar_mul(o_sb[:], o_sb[:], TWO_PI)
            nc.sync.dma_start(
                outT[:, :].rearrange("(m p) b -> p m b", p=P),
                o_sb[:].rearrange("p (m b) -> p m b", b=B),
            )
            pp_cm.__exit__(None, None, None)

    nc.finalize()
    return nc


def _run(inputs, cfg, trace=False):
    x = np.asarray(inputs["x"], np.float32)
    ei = np.asarray(inputs["edge_index"])
    batch = np.asarray(inputs["batch"]).astype(np.int64)
    W_embed = np.asarray(inputs["W_embed"], np.float32)
    b_embed = np.asarray(inputs["b_embed"], np.float32)
    Wm = np.asarray(inputs["Wm"], np.float32)
    bm = np.asarray(inputs["bm"], np.float32)
    W1 = np.asarray(inputs["W1"], np.float32)
    b1 = np.asarray(inputs["b1"], np.float32)
    W2 = np.asarray(inputs["W2"], np.float32)
    b2 = np.asarray(inputs["b2"], np.float32)

    src = np.asarray(ei[0], np.int64)
    tgt = np.asarray(ei[1], np.int64)
    NS, NSP, B, M = cfg.ns, cfg.ns_pad, cfg.B, cfg.M
    NBLK = NSP // 128

    gidx, sidx, calls = _pack_edges(cfg, src, tgt)

    deg = np.bincount(tgt, minlength=cfg.N).astype(np.float32)
    counts = np.bincount(batch, minlength=B).astype(np.float32)
    invc = 1.0 / np.clip(counts, 1.0, None)

    in_maps = []
    for c in range(NCORES):
        sl = slice(c * NS, (c + 1) * NS)
        x_c = np.zeros((NSP, H), np.float32)
        x_c[:NS] = x[sl]
        deg_c = np.zeros(NSP, np.float32)
        deg_c[:NS] = deg[sl]
        ind_c = np.zeros((NSP, B), np.float32)
        ind_c[np.arange(NS), batch[sl]] = invc[batch[sl]]
        in_maps.append(
            {
                "x_s": x_c,
                "Wemb": W_embed,
                "bemb": b_embed.reshape(H, 1),
                "Wm": Wm,
                "bmv": bm.reshape(cfg.L, 1, H),
                "W1r": W1,
                "b1r": np.ascontiguousarray(b1.reshape(2, H).T),
                "W2r": W2,
                "b2r": np.ascontiguousarray(b2.reshape(M // 128, H).T),
                "gidx": gidx[c],
                "sidx": sidx[c],
                "deg_pb": np.ascontiguousarray(deg_c.reshape(NBLK, P).T),
                "degT": deg_c.reshape(1, NSP),
                "ind": np.ascontiguousarray(
                    ind_c.reshape(NBLK, P, B).transpose(1, 0, 2).reshape(P, NBLK * B)
                ),
            }
        )

    TOT = gidx.shape[2]
    nc = _build(cfg, TOT, calls)
    res = run_bass_kernel_spmd(
        nc, in_maps, core_ids=list(range(NCORES)), trace=trace
    )
    out = np.ascontiguousarray(res.results[0]["outT"].T)
    return out, res


def kernel(**inputs) -> np.ndarray:
    cfg = Cfg(N=100000, E=1600000, B=16, M=2048, L=3, chunk=25000, blk=6144)
    trace = bool(os.environ.get("GNN_TRACE"))
    out, _ = _run(inputs, cfg, trace=trace)
    return out



# revision 5
# speedup vs baseline: 1.6164x; 1.6164x over previous
"""Trainium2 Bass kernel: GNN message passing (metasurface inverse design).

Distribution (8 NeuronCores):
  - Edges sharded by target-node range (12500 nodes per core).
  - Node state h replicated each layer via AllGather of the per-core shard.
  - Per-edge gather h[src] uses the SWDGE dma_gather extended instruction
    (512B rows); edges are gathered in (src-chunk, tgt-block, tgt) order.
  - The per-target scatter-add is done on the TensorEngine: for each
    128-edge chunk a one-hot matrix S[j, t] (DVE is_equal against an iota
    row) and matmul(psum[f, t] += E_chunk^T. S) accumulate the segment sum
    per 128-target block.  No SWDGE scatter, no duplicate-target rounds,
    no agg zeroing; the aggregation lands feature-major in PSUM, which is
    exactly the layout the dense phase consumes (saves the agg transposes).

Math restructuring (linearity of the edge matmul over the segment-sum):
    reference: agg = segsum_tgt(concat(h[src], h[tgt]) @ Wm[l] + bm[l])
    here:      G   = segsum_tgt(h[src])            (gather + PE one-hot matmul)
               agg = G @ W1 + deg * (h @ W2) + deg x bm
  with Wm[l] = [W1; W2] and deg = in-degree.  All FLOPs are dense matmuls.
"""

import os

import numpy as np

import concourse.bacc as bacc
import concourse.bass as bass
import concourse.mybir as mybir
import concourse.tile as tile
from concourse.bass_utils import run_bass_kernel_spmd
from concourse.masks import make_identity

F32 = mybir.dt.float32
I16 = mybir.dt.int16
NCORES = 8
H = 128
P = 128
TWO_PI = 6.283185307179586


class Cfg:
    def __init__(self, N, E, B, M, L, chunk=25000, blk=8192, wblk=4):
        assert N % NCORES == 0
        self.N, self.E, self.B, self.M, self.L = N, E, B, M, L
        self.chunk = chunk
        self.wblk = wblk  # tgt blocks (128 nodes) per window == dense tile
        self.ns = N // NCORES
        self.ns_pad = ((self.ns + 511) // 512) * 512
        self.nk = N // chunk  # src chunks (int16 gather index range)
        assert chunk <= 32768


def _pack_v2(cfg, src, tgt):
    """Slot layout: per core, stream-major (src chunk k), then tgt block b,
    then tgt.  Each (k, b) group padded to a multiple of 128 slots; group
    sizes are max over cores (SPMD: identical program on all cores).

    Returns gidx [NCORES, 128, TOT//16] int16 (gather indices, 16-wrapped),
    tcol [NCORES, 128, TOT//128] f32 (per chunk, per lane: tgt col within
    block, or -1 for padding), windows (shared call/chunk structure), and
    rowcap (max 128-slot rows per gather call).
    """
    NS, CH, NK, WB = cfg.ns, cfg.chunk, cfg.nk, cfg.wblk
    NBLK = cfg.ns_pad // 128
    core = tgt // NS
    lt = (tgt - core * NS).astype(np.int64)
    k = src // CH
    ls = (src - k * CH).astype(np.int16)
    b = lt // 128

    key = (core * NK + k) * NBLK + b
    cnt = np.bincount(key, minlength=NCORES * NK * NBLK).reshape(NCORES, NK, NBLK)
    nmax = cnt.max(axis=0)  # [NK, NBLK]
    Pkb = ((nmax + 127) // 128) * 128
    Pkb[0, Pkb.sum(axis=0) == 0] = 128  # every block gets >= 1 chunk

    S_k = Pkb.sum(axis=1)
    base_k = np.concatenate([[0], np.cumsum(S_k)[:-1]])
    off = base_k[:, None] + np.concatenate(
        [np.zeros((NK, 1), np.int64), np.cumsum(Pkb, axis=1)[:, :-1]], axis=1
    )
    TOT = int(S_k.sum())

    o = np.lexsort((lt, key))
    st = np.concatenate([[0], np.cumsum(cnt.reshape(-1))[:-1]])
    rank = np.empty(len(tgt), np.int64)
    rank[o] = np.arange(len(tgt)) - st[key[o]]
    slot = off[k, b] + rank

    gidx = np.zeros((NCORES, 16, TOT // 16), np.int16)
    gidx[core, slot % 16, slot // 16] = ls
    gidx = np.tile(gidx, (1, 8, 1))
    tcol = np.full((NCORES, 128, TOT // 128), -1.0, np.float32)
    tcol[core, slot % 128, slot // 128] = (lt - b * 128).astype(np.float32)

    NW = NBLK // WB
    windows = []
    for w in range(NW):
        blocks = list(range(w * WB, (w + 1) * WB))
        per_k = []
        for kk in range(NK):
            s0 = int(off[kk, blocks[0]])
            nsl = int(Pkb[kk, blocks].sum())
            per_k.append((s0, nsl))
        blist = []
        for bb in blocks:
            ch = []
            for kk in range(NK):
                s0k = per_k[kk][0]
                g0 = int(off[kk, bb])
                for i in range(int(Pkb[kk, bb]) // 128):
                    ch.append((kk, (g0 - s0k) // 128 + i, g0 // 128 + i))
            blist.append(ch)
        windows.append((per_k, blist))
    rowcap = max(nsl for per_k, _ in windows for _, nsl in per_k) // 128
    return gidx, tcol, windows, rowcap, TOT


def _build(cfg, TOT, windows, rowcap):
    """Emit the Bass/Tile program (identical for all cores)."""
    phase = os.environ.get("GNN_PHASE", "full")
    NS, NSP, B, M, L = cfg.ns, cfg.ns_pad, cfg.B, cfg.M, cfg.L
    NT = NSP // 512  # dense tiles (512 nodes each)
    NBLK = NSP // 128
    MT = M // 128
    NK = cfg.nk
    assert len(windows) == NT and cfg.wblk == 4

    nc = bacc.Bacc(None, num_devices=NCORES)

    x_s = nc.dram_tensor("x_s", [NSP, H], F32, kind="ExternalInput")
    Wemb = nc.dram_tensor("Wemb", [H, H], F32, kind="ExternalInput")
    bemb = nc.dram_tensor("bemb", [H, 1], F32, kind="ExternalInput")
    Wm = nc.dram_tensor("Wm", [L, 2 * H, H], F32, kind="ExternalInput")
    bmv = nc.dram_tensor("bmv", [L, 1, H], F32, kind="ExternalInput")
    W1r = nc.dram_tensor("W1r", [H, 2 * H], F32, kind="ExternalInput")
    b1r = nc.dram_tensor("b1r", [H, 2], F32, kind="ExternalInput")
    W2r = nc.dram_tensor("W2r", [2 * H, M], F32, kind="ExternalInput")
    b2r = nc.dram_tensor("b2r", [H, MT], F32, kind="ExternalInput")
    gidx = nc.dram_tensor("gidx", [P, TOT // 16], I16, kind="ExternalInput")
    tcolT = nc.dram_tensor("tcolT", [P, TOT // 128], F32, kind="ExternalInput")
    iota128 = nc.dram_tensor("iota128", [P, P], F32, kind="ExternalInput")
    deg_pb = nc.dram_tensor("deg_pb", [P, NBLK], F32, kind="ExternalInput")
    degT = nc.dram_tensor("degT", [1, NSP], F32, kind="ExternalInput")
    ind = nc.dram_tensor("ind", [P, NBLK * B], F32, kind="ExternalInput")

    outT = nc.dram_tensor("outT", [M, B], F32, kind="ExternalOutput")

    h_full = nc.dram_tensor("h_full", [cfg.N, H], F32, addr_space="Shared")
    h_sh = nc.dram_tensor("h_sh", [NSP, H], F32)
    gpart = nc.dram_tensor("gpart", [H, B], F32)
    gsum = nc.dram_tensor("gsum", [H, B], F32, addr_space="Shared")

    groups = [list(range(NCORES))]

    with tile.TileContext(nc) as tc:
        with (
            tc.tile_pool(name="const", bufs=1) as cp,
            tc.tile_pool(name="work", bufs=2) as wp,
            tc.tile_pool(name="sgen", bufs=4) as sp,
        ):
            pp_cm = tc.tile_pool(name="psum", bufs=2, space="PSUM")
            pp = pp_cm.__enter__()
            ep_cm = tc.tile_pool(name="edge", bufs=2)
            ep = ep_cm.__enter__()
            # ---- persistent constants in SBUF ----
            ident = cp.tile([P, P], F32, tag="ident")
            make_identity(nc, ident[:])
            Wemb_sb = cp.tile([H, H], F32, tag="wemb")
            nc.sync.dma_start(Wemb_sb[:], Wemb[:])
            bemb_sb = cp.tile([H, 1], F32, tag="bemb")
            nc.sync.dma_start(bemb_sb[:], bemb[:])
            W1_sb, W2_sb, bm_sb = [], [], []
            for l in range(L):
                w1 = cp.tile([H, H], F32, tag=f"w1_{l}")
                nc.sync.dma_start(w1[:], Wm[l, :H, :])
                w2 = cp.tile([H, H], F32, tag=f"w2_{l}")
                nc.sync.dma_start(w2[:], Wm[l, H:, :])
                bm_t = cp.tile([1, H], F32, tag=f"bm_{l}")
                nc.sync.dma_start(bm_t[:], bmv[l, :, :])
                W1_sb.append(w1)
                W2_sb.append(w2)
                bm_sb.append(bm_t)
            deg_sb = cp.tile([P, NBLK], F32, tag="deg")
            nc.sync.dma_start(deg_sb[:], deg_pb[:])
            tcol_sb = cp.tile([P, TOT // 128], F32, tag="tcol")
            nc.sync.dma_start(tcol_sb[:], tcolT[:])
            iota_sb = cp.tile([P, P], F32, tag="iota")
            nc.sync.dma_start(iota_sb[:], iota128[:])

            def load_nm(pool, dram_rows, tag):
                """Load 512 node-major rows as [128, 4, 128] (block, feat)."""
                t = pool.tile([P, 4 * H], F32, tag=tag)
                nc.sync.dma_start(
                    t[:].rearrange("p (b f) -> p b f", f=H),
                    dram_rows.rearrange("(b p) f -> p b f", p=P),
                )
                return t

            def store_nm(t, dram_rows):
                nc.sync.dma_start(
                    dram_rows.rearrange("(b p) f -> p b f", p=P),
                    t[:].rearrange("p (b f) -> p b f", f=H),
                )

            def transpose4(src_sb, tag):
                """4x PE transpose of [128,(4,128)] blocks -> PSUM [128,512]."""
                ps = pp.tile([P, 512], F32, tag=tag)
                for b in range(4):
                    nc.tensor.transpose(
                        ps[:, b * H : (b + 1) * H],
                        src_sb[:, b * H : (b + 1) * H],
                        ident[:],
                    )
                return ps

            # ---- embedding: h0 = x @ Wemb + bemb (own shard) ----
            for t in range(NT):
                rows = slice(512 * t, 512 * (t + 1))
                xt = load_nm(wp, x_s[rows, :], "in_a")
                ps_x = transpose4(xt, "ps_b")
                xT = wp.tile([P, 512], F32, tag="t_a")
                nc.vector.tensor_copy(xT[:], ps_x[:])
                ps_h = pp.tile([P, 512], F32, tag="ps_mm")
                nc.tensor.matmul(ps_h[:], Wemb_sb[:], xT[:], start=True, stop=True)
                hT = wp.tile([P, 512], F32, tag="t_b")
                nc.vector.tensor_scalar_add(hT[:], ps_h[:], bemb_sb[:, 0:1])
                ps_nm = transpose4(hT, "ps_nm")
                hn = wp.tile([P, 4 * H], F32, tag="out_a")
                nc.vector.tensor_copy(hn[:], ps_nm[:])
                store_nm(hn, h_sh[rows, :])

            nc.gpsimd.collective_compute(
                "AllGather",
                mybir.AluOpType.bypass,
                replica_groups=groups,
                ins=[h_sh[0:NS, :]],
                outs=[h_full[:, :]],
            )

            # ---- message-passing layers ----
            for l in range(L):
                for w, (per_k, blist) in enumerate(windows):
                    # gather h[src] for this window, per src-chunk stream
                    ets = [None] * NK
                    if phase != "noedge":
                        for kk, (s0, nsl) in enumerate(per_k):
                            if nsl == 0:
                                continue
                            gi = sp.tile([P, rowcap * 8], I16, tag=f"gi{kk}")
                            nc.scalar.dma_start(
                                gi[:, : nsl // 16],
                                gidx[:, s0 // 16 : (s0 + nsl) // 16],
                            )
                            et = ep.tile([P, rowcap * H], F32, tag=f"et{kk}")
                            ets[kk] = et
                            nc.gpsimd.dma_gather(
                                et[:].rearrange("p (r f) -> p r f", f=H)[
                                    :, : nsl // 128, :
                                ],
                                h_full[kk * cfg.chunk : (kk + 1) * cfg.chunk, :],
                                gi[:, : nsl // 16],
                                nsl,
                                nsl,
                                H,
                                single_packet=False,
                            )

                    # segment-sum on PE: psum[f, t] over the window's 4 blocks
                    ps_seg = pp.tile([P, 512], F32, tag="seg")
                    if phase == "noedge":
                        nc.vector.memset(ps_seg[:], 0.0)
                    else:
                        for j, ch in enumerate(blist):
                            nch = len(ch)
                            for i, (kk, r, gc) in enumerate(ch):
                                S = sp.tile([P, H], F32, tag="s")
                                nc.vector.tensor_scalar(
                                    S[:],
                                    iota_sb[:],
                                    tcol_sb[:, gc : gc + 1],
                                    None,
                                    mybir.AluOpType.is_equal,
                                )
                                ev = ets[kk][:].rearrange("p (r f) -> p r f", f=H)
                                nc.tensor.matmul(
                                    ps_seg[:, j * H : (j + 1) * H],
                                    ev[:, r, :],
                                    S[:],
                                    start=(i == 0),
                                    stop=(i == nch - 1),
                                )
                    gT = wp.tile([P, 512], F32, tag="t_a")
                    nc.vector.tensor_copy(gT[:], ps_seg[:])

                    # dense: h = relu(h + G @ W1 + deg*(h @ W2) + deg x bm)
                    if phase != "nodense":
                        rows = slice(512 * w, 512 * (w + 1))
                        ht = load_nm(wp, h_sh[rows, :], "in_b")
                        degT_t = wp.tile([1, 512], F32, tag="degt")
                        nc.sync.dma_start(degT_t[:], degT[:, rows])
                        dh = wp.tile([P, 4 * H], F32, tag="dh")
                        for b in range(4):
                            bs = slice(b * H, (b + 1) * H)
                            nc.vector.tensor_scalar_mul(
                                dh[:, bs],
                                ht[:, bs],
                                deg_sb[:, 4 * w + b : 4 * w + b + 1],
                            )
                        ps_d = transpose4(dh, "ps_b")
                        dhT = wp.tile([P, 512], F32, tag="t_b")
                        nc.vector.tensor_copy(dhT[:], ps_d[:])
                        ps_mm = pp.tile([P, 512], F32, tag="ps_mm")
                        nc.tensor.matmul(
                            ps_mm[:], W1_sb[l][:], gT[:], start=True, stop=False
                        )
                        nc.tensor.matmul(
                            ps_mm[:], W2_sb[l][:], dhT[:], start=False, stop=False
                        )
                        nc.tensor.matmul(
                            ps_mm[:], bm_sb[l][:], degT_t[:], start=False, stop=True
                        )
                        mmT = wp.tile([P, 512], F32, tag="t_c")
                        nc.vector.tensor_copy(mmT[:], ps_mm[:])
                        ps_nm = transpose4(mmT, "ps_nm")
                        hn = wp.tile([P, 4 * H], F32, tag="out_a")
                        nc.vector.tensor_add(hn[:], ht[:], ps_nm[:])
                        nc.vector.tensor_scalar_max(hn[:], hn[:], 0.0)
                        store_nm(hn, h_sh[rows, :])

                if l < L - 1:
                    nc.gpsimd.collective_compute(
                        "AllGather",
                        mybir.AluOpType.bypass,
                        replica_groups=groups,
                        ins=[h_sh[0:NS, :]],
                        outs=[h_full[:, :]],
                    )

            # ---- readout: g = per-graph mean (ind already holds 1/count) ----
            ep_cm.__exit__(None, None, None)
            pp_cm.__exit__(None, None, None)
            pp_cm = tc.tile_pool(name="psum_ro", bufs=1, space="PSUM")
            pp = pp_cm.__enter__()
            ro_cm = tc.tile_pool(name="readout", bufs=1)
            ro = ro_cm.__enter__()
            ind_sb = ro.tile([P, NBLK * B], F32, tag="ind")
            nc.sync.dma_start(ind_sb[:], ind[:])
            ps_gr = pp.tile([P, B], F32, tag="ps_gr")
            for t in range(NT):
                rows = slice(512 * t, 512 * (t + 1))
                hro = load_nm(wp, h_sh[rows, :], "in_a")
                for b in range(4):
                    blk = 4 * t + b
                    nc.tensor.matmul(
                        ps_gr[:],
                        hro[:, b * H : (b + 1) * H],
                        ind_sb[:, blk * B : (blk + 1) * B],
                        start=(blk == 0),
                        stop=(blk == NBLK - 1),
                    )
            gp_sb = wp.tile([P, B], F32, tag="gp")
            nc.vector.tensor_copy(gp_sb[:], ps_gr[:])
            nc.sync.dma_start(gpart[:, :], gp_sb[:])
            nc.gpsimd.collective_compute(
                "AllReduce",
                mybir.AluOpType.add,
                replica_groups=groups,
                ins=[gpart[:, :]],
                outs=[gsum[:, :]],
            )
            gs_sb = wp.tile([P, B], F32, tag="gs")
            nc.sync.dma_start(gs_sb[:], gsum[:, :])

            W1r_sb = ro.tile([H, 2 * H], F32, tag="w1r")
            nc.sync.dma_start(W1r_sb[:], W1r[:])
            b1_sb = ro.tile([H, 2], F32, tag="b1r")
            nc.sync.dma_start(b1_sb[:], b1r[:])
            W2ra_sb = ro.tile([H, M], F32, tag="w2ra")
            nc.sync.dma_start(W2ra_sb[:], W2r[0:H, :])
            W2rb_sb = ro.tile([H, M], F32, tag="w2rb")
            nc.sync.dma_start(W2rb_sb[:], W2r[H:, :])
            b2_sb = ro.tile([H, MT], F32, tag="b2r")
            nc.sync.dma_start(b2_sb[:], b2r[:])

            z1 = []
            for i in range(2):
                ps_z = pp.tile([P, B], F32, tag=f"ps_z{i}")
                nc.tensor.matmul(
                    ps_z[:],
                    W1r_sb[:, i * H : (i + 1) * H],
                    gs_sb[:],
                    start=True,
                    stop=True,
                )
                zt = wp.tile([P, B], F32, tag=f"z1_{i}")
                nc.vector.tensor_scalar(
                    zt[:],
                    ps_z[:],
                    b1_sb[:, i : i + 1],
                    0.0,
                    mybir.AluOpType.add,
                    mybir.AluOpType.max,
                )
                z1.append(zt)

            o_sb = wp.tile([P, MT * B], F32, tag="o")
            for m in range(MT):
                ps_o = pp.tile([P, B], F32, tag="ps_o")
                ms = slice(m * H, (m + 1) * H)
                nc.tensor.matmul(
                    ps_o[:], W2ra_sb[:, ms], z1[0][:], start=True, stop=False
                )
                nc.tensor.matmul(
                    ps_o[:], W2rb_sb[:, ms], z1[1][:], start=False, stop=True
                )
                nc.scalar.activation(
                    o_sb[:, m * B : (m + 1) * B],
                    ps_o[:],
                    mybir.ActivationFunctionType.Sigmoid,
                    bias=b2_sb[:, m : m + 1],
                    scale=1.0,
                )
            nc.vector.tensor_scalar_mul(o_sb[:], o_sb[:], TWO_PI)
            nc.sync.dma_start(
                outT[:, :].rearrange("(m p) b -> p m b", p=P),
                o_sb[:].rearrange("p (m b) -> p m b", b=B),
            )
            ro_cm.__exit__(None, None, None)
            pp_cm.__exit__(None, None, None)

    nc.finalize()
    return nc


def _run(inputs, cfg, trace=False):
    x = np.asarray(inputs["x"], np.float32)
    ei = np.asarray(inputs["edge_index"])
    batch = np.asarray(inputs["batch"]).astype(np.int64)
    W_embed = np.asarray(inputs["W_embed"], np.float32)
    b_embed = np.asarray(inputs["b_embed"], np.float32)
    Wm = np.asarray(inputs["Wm"], np.float32)
    bm = np.asarray(inputs["bm"], np.float32)
    W1 = np.asarray(inputs["W1"], np.float32)
    b1 = np.asarray(inputs["b1"], np.float32)
    W2 = np.asarray(inputs["W2"], np.float32)
    b2 = np.asarray(inputs["b2"], np.float32)

    src = np.asarray(ei[0], np.int64)
    tgt = np.asarray(ei[1], np.int64)
    NS, NSP, B, M = cfg.ns, cfg.ns_pad, cfg.B, cfg.M
    NBLK = NSP // 128

    gidx, tcol, windows, rowcap, TOT = _pack_v2(cfg, src, tgt)

    deg = np.bincount(tgt, minlength=cfg.N).astype(np.float32)
    counts = np.bincount(batch, minlength=B).astype(np.float32)
    invc = 1.0 / np.clip(counts, 1.0, None)
    iota = np.broadcast_to(
        np.arange(P, dtype=np.float32)[None, :], (P, P)
    ).copy()

    in_maps = []
    for c in range(NCORES):
        sl = slice(c * NS, (c + 1) * NS)
        x_c = np.zeros((NSP, H), np.float32)
        x_c[:NS] = x[sl]
        deg_c = np.zeros(NSP, np.float32)
        deg_c[:NS] = deg[sl]
        ind_c = np.zeros((NSP, B), np.float32)
        ind_c[np.arange(NS), batch[sl]] = invc[batch[sl]]
        in_maps.append(
            {
                "x_s": x_c,
                "Wemb": W_embed,
                "bemb": b_embed.reshape(H, 1),
                "Wm": Wm,
                "bmv": bm.reshape(cfg.L, 1, H),
                "W1r": W1,
                "b1r": np.ascontiguousarray(b1.reshape(2, H).T),
                "W2r": W2,
                "b2r": np.ascontiguousarray(b2.reshape(M // 128, H).T),
                "gidx": gidx[c],
                "tcolT": tcol[c],
                "iota128": iota,
                "deg_pb": np.ascontiguousarray(deg_c.reshape(NBLK, P).T),
                "degT": deg_c.reshape(1, NSP),
                "ind": np.ascontiguousarray(
                    ind_c.reshape(NBLK, P, B).transpose(1, 0, 2).reshape(P, NBLK * B)
                ),
            }
        )

    nc = _build(cfg, TOT, windows, rowcap)
    res = run_bass_kernel_spmd(
        nc, in_maps, core_ids=list(range(NCORES)), trace=trace
    )
    out = np.ascontiguousarray(res.results[0]["outT"].T)
    return out, res


def kernel(**inputs) -> np.ndarray:
    cfg = Cfg(N=100000, E=1600000, B=16, M=2048, L=3, chunk=25000, blk=6144)
    trace = bool(os.environ.get("GNN_TRACE"))
    out, _ = _run(inputs, cfg, trace=trace)
    return out


# revision 7
# speedup vs baseline: 1.9210x; 1.1885x over previous
"""Trainium2 Bass kernel: GNN message passing (metasurface inverse design).

Distribution (8 NeuronCores):
  - Edges sharded by target-node range (12500 nodes per core).
  - Node state h replicated each layer via AllGather of the per-core shard.
  - Per-edge gather h[src] uses the SWDGE dma_gather extended instruction
    (512B rows); edges are gathered in (src-chunk, tgt-block, tgt) order.
  - The per-target scatter-add is done on the TensorEngine: for each
    128-edge chunk a one-hot matrix S[j, t] (DVE is_equal against an iota
    row) and matmul(psum[f, t] += E_chunk^T. S) accumulate the segment sum
    per 128-target block.  No SWDGE scatter, no duplicate-target rounds,
    no agg zeroing; the aggregation lands feature-major in PSUM, which is
    exactly the layout the dense phase consumes (saves the agg transposes).

Math restructuring (linearity of the edge matmul over the segment-sum):
    reference: agg = segsum_tgt(concat(h[src], h[tgt]) @ Wm[l] + bm[l])
    here:      G   = segsum_tgt(h[src])            (gather + PE one-hot matmul)
               agg = G @ W1 + deg * (h @ W2) + deg x bm
  with Wm[l] = [W1; W2] and deg = in-degree.  All FLOPs are dense matmuls.
"""

import os

import numpy as np

import concourse.bacc as bacc
import concourse.bass as bass
import concourse.mybir as mybir
import concourse.tile as tile
from concourse.bass_utils import run_bass_kernel_spmd
from concourse.masks import make_identity

F32 = mybir.dt.float32
I16 = mybir.dt.int16
NCORES = 8
H = 128
P = 128
TWO_PI = 6.283185307179586


class Cfg:
    def __init__(self, N, E, B, M, L, chunk=25000, blk=8192, wblk=4):
        assert N % NCORES == 0
        self.N, self.E, self.B, self.M, self.L = N, E, B, M, L
        self.chunk = chunk
        self.wblk = wblk  # tgt blocks (128 nodes) per window == dense tile
        self.ns = N // NCORES
        self.ns_pad = ((self.ns + 511) // 512) * 512
        self.nk = N // chunk  # src chunks (int16 gather index range)
        assert chunk <= 32768


def _pack_v2(cfg, src, tgt):
    """Slot layout: per core, stream-major (src chunk k), then tgt block b,
    then tgt.  Each (k, b) group padded to a multiple of 128 slots; group
    sizes are max over cores (SPMD: identical program on all cores).

    Returns gidx [NCORES, 128, TOT//16] int16 (gather indices, 16-wrapped),
    tcol [NCORES, 128, TOT//128] f32 (per chunk, per lane: tgt col within
    block, or -1 for padding), windows (shared call/chunk structure), and
    rowcap (max 128-slot rows per gather call).
    """
    NS, CH, NK, WB = cfg.ns, cfg.chunk, cfg.nk, cfg.wblk
    NBLK = cfg.ns_pad // 128
    core = tgt // NS
    lt = (tgt - core * NS).astype(np.int64)
    k = src // CH
    ls = (src - k * CH).astype(np.int16)
    b = lt // 128

    key = (core * NK + k) * NBLK + b
    cnt = np.bincount(key, minlength=NCORES * NK * NBLK).reshape(NCORES, NK, NBLK)
    nmax = cnt.max(axis=0)  # [NK, NBLK]
    Pkb = ((nmax + 127) // 128) * 128
    Pkb[0, Pkb.sum(axis=0) == 0] = 128  # every block gets >= 1 chunk

    S_k = Pkb.sum(axis=1)
    base_k = np.concatenate([[0], np.cumsum(S_k)[:-1]])
    off = base_k[:, None] + np.concatenate(
        [np.zeros((NK, 1), np.int64), np.cumsum(Pkb, axis=1)[:, :-1]], axis=1
    )
    TOT = int(S_k.sum())

    o = np.lexsort((lt, key))
    st = np.concatenate([[0], np.cumsum(cnt.reshape(-1))[:-1]])
    rank = np.empty(len(tgt), np.int64)
    rank[o] = np.arange(len(tgt)) - st[key[o]]
    slot = off[k, b] + rank

    gidx = np.zeros((NCORES, 16, TOT // 16), np.int16)
    gidx[core, slot % 16, slot // 16] = ls
    gidx = np.tile(gidx, (1, 8, 1))
    tcol = np.full((NCORES, 128, TOT // 128), -1.0, np.float32)
    tcol[core, slot % 128, slot // 128] = (lt - b * 128).astype(np.float32)

    NW = NBLK // WB
    windows = []
    for w in range(NW):
        blocks = list(range(w * WB, (w + 1) * WB))
        per_k = []
        for kk in range(NK):
            s0 = int(off[kk, blocks[0]])
            nsl = int(Pkb[kk, blocks].sum())
            per_k.append((s0, nsl))
        blist = []
        for bb in blocks:
            ch = []
            for kk in range(NK):
                s0k = per_k[kk][0]
                g0 = int(off[kk, bb])
                for i in range(int(Pkb[kk, bb]) // 128):
                    ch.append((kk, (g0 - s0k) // 128 + i, g0 // 128 + i))
            blist.append(ch)
        windows.append((per_k, blist))
    rowcap = max(nsl for per_k, _ in windows for _, nsl in per_k) // 128
    return gidx, tcol, windows, rowcap, TOT


def _build(cfg, TOT, windows, rowcap):
    """Emit the Bass/Tile program (identical for all cores)."""
    phase = os.environ.get("GNN_PHASE", "full")
    NS, NSP, B, M, L = cfg.ns, cfg.ns_pad, cfg.B, cfg.M, cfg.L
    NT = NSP // 512  # dense tiles (512 nodes each)
    NBLK = NSP // 128
    MT = M // 128
    NK = cfg.nk
    assert len(windows) == NT and cfg.wblk == 4

    nqueues = int(os.environ.get("GNN_QUEUES", "1"))
    nc = bacc.Bacc(None, num_devices=NCORES, num_swdge_queues=nqueues)

    x_s = nc.dram_tensor("x_s", [NSP, H], F32, kind="ExternalInput")
    Wemb = nc.dram_tensor("Wemb", [H, H], F32, kind="ExternalInput")
    bemb = nc.dram_tensor("bemb", [H, 1], F32, kind="ExternalInput")
    Wm = nc.dram_tensor("Wm", [L, 2 * H, H], F32, kind="ExternalInput")
    bmv = nc.dram_tensor("bmv", [L, 1, H], F32, kind="ExternalInput")
    W1r = nc.dram_tensor("W1r", [H, 2 * H], F32, kind="ExternalInput")
    b1r = nc.dram_tensor("b1r", [H, 2], F32, kind="ExternalInput")
    W2r = nc.dram_tensor("W2r", [2 * H, M], F32, kind="ExternalInput")
    b2r = nc.dram_tensor("b2r", [H, MT], F32, kind="ExternalInput")
    gidx = nc.dram_tensor("gidx", [P, TOT // 16], I16, kind="ExternalInput")
    tcolT = nc.dram_tensor("tcolT", [P, TOT // 128], F32, kind="ExternalInput")
    iota128 = nc.dram_tensor("iota128", [P, P], F32, kind="ExternalInput")
    deg_pb = nc.dram_tensor("deg_pb", [P, NBLK], F32, kind="ExternalInput")
    degT = nc.dram_tensor("degT", [1, NSP], F32, kind="ExternalInput")
    ind = nc.dram_tensor("ind", [P, NBLK * B], F32, kind="ExternalInput")

    outT = nc.dram_tensor("outT", [M, B], F32, kind="ExternalOutput")

    h_full = nc.dram_tensor("h_full", [cfg.N, H], F32, addr_space="Shared")
    h_sh = nc.dram_tensor("h_sh", [NSP, H], F32)
    gpart = nc.dram_tensor("gpart", [H, B], F32)
    gsum = nc.dram_tensor("gsum", [H, B], F32, addr_space="Shared")

    groups = [list(range(NCORES))]

    with tile.TileContext(nc) as tc:
        with (
            tc.tile_pool(name="const", bufs=1) as cp,
            tc.tile_pool(name="work", bufs=2) as wp,
            tc.tile_pool(name="sgen", bufs=4) as sp,
        ):
            pp_cm = tc.tile_pool(name="psum", bufs=2, space="PSUM")
            pp = pp_cm.__enter__()
            ep_cm = tc.tile_pool(name="edge", bufs=2)
            ep = ep_cm.__enter__()
            # ---- persistent constants in SBUF ----
            ident = cp.tile([P, P], F32, tag="ident")
            make_identity(nc, ident[:])
            Wemb_sb = cp.tile([H, H], F32, tag="wemb")
            nc.sync.dma_start(Wemb_sb[:], Wemb[:])
            bemb_sb = cp.tile([H, 1], F32, tag="bemb")
            nc.sync.dma_start(bemb_sb[:], bemb[:])
            W1_sb, W2_sb, bm_sb = [], [], []
            for l in range(L):
                w1 = cp.tile([H, H], F32, tag=f"w1_{l}")
                nc.sync.dma_start(w1[:], Wm[l, :H, :])
                w2 = cp.tile([H, H], F32, tag=f"w2_{l}")
                nc.sync.dma_start(w2[:], Wm[l, H:, :])
                bm_t = cp.tile([1, H], F32, tag=f"bm_{l}")
                nc.sync.dma_start(bm_t[:], bmv[l, :, :])
                W1_sb.append(w1)
                W2_sb.append(w2)
                bm_sb.append(bm_t)
            deg_sb = cp.tile([P, NBLK], F32, tag="deg")
            nc.sync.dma_start(deg_sb[:], deg_pb[:])
            tcol_sb = cp.tile([P, TOT // 128], F32, tag="tcol")
            nc.sync.dma_start(tcol_sb[:], tcolT[:])
            iota_sb = cp.tile([P, P], F32, tag="iota")
            nc.sync.dma_start(iota_sb[:], iota128[:])

            def load_nm(pool, dram_rows, tag):
                """Load 512 node-major rows as [128, 4, 128] (block, feat)."""
                t = pool.tile([P, 4 * H], F32, tag=tag)
                nc.sync.dma_start(
                    t[:].rearrange("p (b f) -> p b f", f=H),
                    dram_rows.rearrange("(b p) f -> p b f", p=P),
                )
                return t

            def store_nm(t, dram_rows):
                nc.sync.dma_start(
                    dram_rows.rearrange("(b p) f -> p b f", p=P),
                    t[:].rearrange("p (b f) -> p b f", f=H),
                )

            def transpose4(src_sb, tag):
                """4x PE transpose of [128,(4,128)] blocks -> PSUM [128,512]."""
                ps = pp.tile([P, 512], F32, tag=tag)
                for b in range(4):
                    nc.tensor.transpose(
                        ps[:, b * H : (b + 1) * H],
                        src_sb[:, b * H : (b + 1) * H],
                        ident[:],
                    )
                return ps

            # ---- embedding: h0 = x @ Wemb + bemb (own shard) ----
            for t in range(NT):
                rows = slice(512 * t, 512 * (t + 1))
                xt = load_nm(wp, x_s[rows, :], "in_a")
                ps_x = transpose4(xt, "ps_b")
                xT = wp.tile([P, 512], F32, tag="t_a")
                nc.vector.tensor_copy(xT[:], ps_x[:])
                ps_h = pp.tile([P, 512], F32, tag="ps_mm")
                nc.tensor.matmul(ps_h[:], Wemb_sb[:], xT[:], start=True, stop=True)
                hT = wp.tile([P, 512], F32, tag="t_b")
                nc.vector.tensor_scalar_add(hT[:], ps_h[:], bemb_sb[:, 0:1])
                ps_nm = transpose4(hT, "ps_nm")
                hn = wp.tile([P, 4 * H], F32, tag="out_a")
                nc.vector.tensor_copy(hn[:], ps_nm[:])
                store_nm(hn, h_sh[rows, :])

            nc.gpsimd.collective_compute(
                "AllGather",
                mybir.AluOpType.bypass,
                replica_groups=groups,
                ins=[h_sh[0:NS, :]],
                outs=[h_full[:, :]],
            )

            # ---- message-passing layers ----
            for l in range(L):
                for w, (per_k, blist) in enumerate(windows):
                    # gather h[src] for this window, per src-chunk stream
                    ets = [None] * NK
                    if phase != "noedge":
                        for kk, (s0, nsl) in enumerate(per_k):
                            if nsl == 0:
                                continue
                            gi = sp.tile([P, rowcap * 8], I16, tag=f"gi{kk}")
                            nc.scalar.dma_start(
                                gi[:, : nsl // 16],
                                gidx[:, s0 // 16 : (s0 + nsl) // 16],
                            )
                            et = ep.tile([P, rowcap * H], F32, tag=f"et{kk}")
                            ets[kk] = et
                            nc.gpsimd.dma_gather(
                                et[:].rearrange("p (r f) -> p r f", f=H)[
                                    :, : nsl // 128, :
                                ],
                                h_full[kk * cfg.chunk : (kk + 1) * cfg.chunk, :],
                                gi[:, : nsl // 16],
                                nsl,
                                nsl,
                                H,
                                single_packet=False,
                                queue_num=kk % nqueues,
                            )

                    # segment-sum on PE: psum[f, t] over the window's 4 blocks
                    ps_seg = pp.tile([P, 512], F32, tag="seg")
                    if phase == "noedge":
                        nc.vector.memset(ps_seg[:], 0.0)
                    else:
                        for j, ch in enumerate(blist):
                            nch = len(ch)
                            for i, (kk, r, gc) in enumerate(ch):
                                S = sp.tile([P, H], F32, tag="s")
                                nc.vector.tensor_scalar(
                                    S[:],
                                    iota_sb[:],
                                    tcol_sb[:, gc : gc + 1],
                                    None,
                                    mybir.AluOpType.is_equal,
                                )
                                ev = ets[kk][:].rearrange("p (r f) -> p r f", f=H)
                                nc.tensor.matmul(
                                    ps_seg[:, j * H : (j + 1) * H],
                                    ev[:, r, :],
                                    S[:],
                                    start=(i == 0),
                                    stop=(i == nch - 1),
                                )
                    gT = wp.tile([P, 512], F32, tag="t_a")
                    nc.vector.tensor_copy(gT[:], ps_seg[:])

                    # dense: h = relu(h + G @ W1 + deg*(h @ W2) + deg x bm)
                    if phase != "nodense":
                        rows = slice(512 * w, 512 * (w + 1))
                        ht = load_nm(wp, h_sh[rows, :], "in_b")
                        degT_t = wp.tile([1, 512], F32, tag="degt")
                        nc.sync.dma_start(degT_t[:], degT[:, rows])
                        dh = wp.tile([P, 4 * H], F32, tag="dh")
                        for b in range(4):
                            bs = slice(b * H, (b + 1) * H)
                            nc.vector.tensor_scalar_mul(
                                dh[:, bs],
                                ht[:, bs],
                                deg_sb[:, 4 * w + b : 4 * w + b + 1],
                            )
                        ps_d = transpose4(dh, "ps_b")
                        dhT = wp.tile([P, 512], F32, tag="t_b")
                        nc.vector.tensor_copy(dhT[:], ps_d[:])
                        ps_mm = pp.tile([P, 512], F32, tag="ps_mm")
                        nc.tensor.matmul(
                            ps_mm[:], W1_sb[l][:], gT[:], start=True, stop=False
                        )
                        nc.tensor.matmul(
                            ps_mm[:], W2_sb[l][:], dhT[:], start=False, stop=False
                        )
                        nc.tensor.matmul(
                            ps_mm[:], bm_sb[l][:], degT_t[:], start=False, stop=True
                        )
                        mmT = wp.tile([P, 512], F32, tag="t_c")
                        nc.vector.tensor_copy(mmT[:], ps_mm[:])
                        ps_nm = transpose4(mmT, "ps_nm")
                        hn = wp.tile([P, 4 * H], F32, tag="out_a")
                        nc.vector.tensor_add(hn[:], ht[:], ps_nm[:])
                        nc.vector.tensor_scalar_max(hn[:], hn[:], 0.0)
                        store_nm(hn, h_sh[rows, :])

                if l < L - 1:
                    nc.gpsimd.collective_compute(
                        "AllGather",
                        mybir.AluOpType.bypass,
                        replica_groups=groups,
                        ins=[h_sh[0:NS, :]],
                        outs=[h_full[:, :]],
                    )

            # ---- readout: g = per-graph mean (ind already holds 1/count) ----
            ep_cm.__exit__(None, None, None)
            pp_cm.__exit__(None, None, None)
            pp_cm = tc.tile_pool(name="psum_ro", bufs=1, space="PSUM")
            pp = pp_cm.__enter__()
            ro_cm = tc.tile_pool(name="readout", bufs=1)
            ro = ro_cm.__enter__()
            ind_sb = ro.tile([P, NBLK * B], F32, tag="ind")
            nc.sync.dma_start(ind_sb[:], ind[:])
            ps_gr = pp.tile([P, B], F32, tag="ps_gr")
            for t in range(NT):
                rows = slice(512 * t, 512 * (t + 1))
                hro = load_nm(wp, h_sh[rows, :], "in_a")
                for b in range(4):
                    blk = 4 * t + b
                    nc.tensor.matmul(
                        ps_gr[:],
                        hro[:, b * H : (b + 1) * H],
                        ind_sb[:, blk * B : (blk + 1) * B],
                        start=(blk == 0),
                        stop=(blk == NBLK - 1),
                    )
            gp_sb = wp.tile([P, B], F32, tag="gp")
            nc.vector.tensor_copy(gp_sb[:], ps_gr[:])
            nc.sync.dma_start(gpart[:, :], gp_sb[:])
            nc.gpsimd.collective_compute(
                "AllReduce",
                mybir.AluOpType.add,
                replica_groups=groups,
                ins=[gpart[:, :]],
                outs=[gsum[:, :]],
            )
            gs_sb = wp.tile([P, B], F32, tag="gs")
            nc.sync.dma_start(gs_sb[:], gsum[:, :])

            W1r_sb = ro.tile([H, 2 * H], F32, tag="w1r")
            nc.sync.dma_start(W1r_sb[:], W1r[:])
            b1_sb = ro.tile([H, 2], F32, tag="b1r")
            nc.sync.dma_start(b1_sb[:], b1r[:])
            W2ra_sb = ro.tile([H, M], F32, tag="w2ra")
            nc.sync.dma_start(W2ra_sb[:], W2r[0:H, :])
            W2rb_sb = ro.tile([H, M], F32, tag="w2rb")
            nc.sync.dma_start(W2rb_sb[:], W2r[H:, :])
            b2_sb = ro.tile([H, MT], F32, tag="b2r")
            nc.sync.dma_start(b2_sb[:], b2r[:])

            z1 = []
            for i in range(2):
                ps_z = pp.tile([P, B], F32, tag=f"ps_z{i}")
                nc.tensor.matmul(
                    ps_z[:],
                    W1r_sb[:, i * H : (i + 1) * H],
                    gs_sb[:],
                    start=True,
                    stop=True,
                )
                zt = wp.tile([P, B], F32, tag=f"z1_{i}")
                nc.vector.tensor_scalar(
                    zt[:],
                    ps_z[:],
                    b1_sb[:, i : i + 1],
                    0.0,
                    mybir.AluOpType.add,
                    mybir.AluOpType.max,
                )
                z1.append(zt)

            o_sb = wp.tile([P, MT * B], F32, tag="o")
            for m in range(MT):
                ps_o = pp.tile([P, B], F32, tag="ps_o")
                ms = slice(m * H, (m + 1) * H)
                nc.tensor.matmul(
                    ps_o[:], W2ra_sb[:, ms], z1[0][:], start=True, stop=False
                )
                nc.tensor.matmul(
                    ps_o[:], W2rb_sb[:, ms], z1[1][:], start=False, stop=True
                )
                nc.scalar.activation(
                    o_sb[:, m * B : (m + 1) * B],
                    ps_o[:],
                    mybir.ActivationFunctionType.Sigmoid,
                    bias=b2_sb[:, m : m + 1],
                    scale=1.0,
                )
            nc.vector.tensor_scalar_mul(o_sb[:], o_sb[:], TWO_PI)
            nc.sync.dma_start(
                outT[:, :].rearrange("(m p) b -> p m b", p=P),
                o_sb[:].rearrange("p (m b) -> p m b", b=B),
            )
            ro_cm.__exit__(None, None, None)
            pp_cm.__exit__(None, None, None)

    nc.finalize()
    return nc


def _run(inputs, cfg, trace=False):
    x = np.asarray(inputs["x"], np.float32)
    ei = np.asarray(inputs["edge_index"])
    batch = np.asarray(inputs["batch"]).astype(np.int64)
    W_embed = np.asarray(inputs["W_embed"], np.float32)
    b_embed = np.asarray(inputs["b_embed"], np.float32)
    Wm = np.asarray(inputs["Wm"], np.float32)
    bm = np.asarray(inputs["bm"], np.float32)
    W1 = np.asarray(inputs["W1"], np.float32)
    b1 = np.asarray(inputs["b1"], np.float32)
    W2 = np.asarray(inputs["W2"], np.float32)
    b2 = np.asarray(inputs["b2"], np.float32)

    src = np.asarray(ei[0], np.int64)
    tgt = np.asarray(ei[1], np.int64)
    NS, NSP, B, M = cfg.ns, cfg.ns_pad, cfg.B, cfg.M
    NBLK = NSP // 128

    gidx, tcol, windows, rowcap, TOT = _pack_v2(cfg, src, tgt)

    deg = np.bincount(tgt, minlength=cfg.N).astype(np.float32)
    counts = np.bincount(batch, minlength=B).astype(np.float32)
    invc = 1.0 / np.clip(counts, 1.0, None)
    iota = np.broadcast_to(
        np.arange(P, dtype=np.float32)[None, :], (P, P)
    ).copy()

    in_maps = []
    for c in range(NCORES):
        sl = slice(c * NS, (c + 1) * NS)
        x_c = np.zeros((NSP, H), np.float32)
        x_c[:NS] = x[sl]
        deg_c = np.zeros(NSP, np.float32)
        deg_c[:NS] = deg[sl]
        ind_c = np.zeros((NSP, B), np.float32)
        ind_c[np.arange(NS), batch[sl]] = invc[batch[sl]]
        in_maps.append(
            {
                "x_s": x_c,
                "Wemb": W_embed,
                "bemb": b_embed.reshape(H, 1),
                "Wm": Wm,
                "bmv": bm.reshape(cfg.L, 1, H),
                "W1r": W1,
                "b1r": np.ascontiguousarray(b1.reshape(2, H).T),
                "W2r": W2,
                "b2r": np.ascontiguousarray(b2.reshape(M // 128, H).T),
                "gidx": gidx[c],
                "tcolT": tcol[c],
                "iota128": iota,
                "deg_pb": np.ascontiguousarray(deg_c.reshape(NBLK, P).T),
                "degT": deg_c.reshape(1, NSP),
                "ind": np.ascontiguousarray(
                    ind_c.reshape(NBLK, P, B).transpose(1, 0, 2).reshape(P, NBLK * B)
                ),
            }
        )

    nc = _build(cfg, TOT, windows, rowcap)
    res = run_bass_kernel_spmd(
        nc, in_maps, core_ids=list(range(NCORES)), trace=trace
    )
    out = np.ascontiguousarray(res.results[0]["outT"].T)
    return out, res


def kernel(**inputs) -> np.ndarray:
    cfg = Cfg(N=100000, E=1600000, B=16, M=2048, L=3, chunk=25000, blk=6144)
    trace = bool(os.environ.get("GNN_TRACE"))
    out, _ = _run(inputs, cfg, trace=trace)
    return out
